# revision 7
# baseline (speedup 1.0000x reference)
"""GNN message-passing kernel for Trainium2 (8 NeuronCores, SPMD).

Reference computation (B=1, N=20000, K=32, D=128, DEPTH=3):
    h0 = graph
    for t in 1..2:
        g[n]  = mean_k h_{t-1}[adj[k, n]]        (neighbor gather + mean)
        h_t   = relu(g @ W[t] + b[t])
    out = stack([h0, h1, h2])                     # [1, 3, N, D]

(matmul and mean commute, so we gather+sum first and multiply once per
 node; the 1/K is folded into W host-side.)

Distribution: nodes sharded 2500/core (padded to 2560 = 20 chunks of 128).
Gather sources are kept TRANSPOSED (feature-major) in SBUF so the
neighbor gather runs on the GPSIMD compute path (`ap_gather`,
out = in[:, idxs]) instead of per-row DMA descriptors. ap_gather moves
16 partitions x 4B per index per Q7 core at ~30ns/idx + ~90us/call, so:

Layer 1 (source = input h0, packed host-side): the 4B word on partition
p holds the bf16 pair (h0[n, 2*(p%64)], h0[n, 2*(p%64)+1]); partitions
0-63 and 64-127 hold identical data, but Q7 cores 0-3 and 4-7 read
different index lists, so ONE 8192-idx call gathers FOUR chunks (512
nodes): chunk-pair A on the lower partition half, B on the upper.
    GT[p, i]      = srcP[p, idxA/B[i]]           (ap_gather, raw 4B)
    gsum[p, n, f] = sum_k GT_bf16[p, 32n+k, f]   (DVE reduce, fp32 out)
    gt[p, f, n]   = bf16(gsum)                   (ACT copy, deinterleave)
    phh_A[e, n]   = sum_f sum_p W1'[2p+f, e] gt[p, f, n]   (2 matmuls,
                    64-partition contraction; same for B on upper half)
    hT[:, n]      = relu(phh + b)                (DVE add + max)

Layer 2 (source = all-gathered h1, fp32 feature-major): plain d=1
gather, two chunks per 8192-idx call:
    GT[d, i]   = srcT[d, idx[i]];  gsum[d, n] = sum_k GT[d, 32n+k]
    phhT[e, n] = sum_d W2'[d, e] bf16(gsum)[d, n];  relu(+b)

Between layers: h1T -> DRAM -> AllGather(fp32) -> 8 block loads into the
reused source tile. Outputs are feature-major [128, NSP]; the host
transposes back.
"""

import numpy as np

import concourse.bacc as bacc
import concourse.mybir as mybir
import concourse.tile as tile
from concourse.bass_utils import run_bass_kernel_spmd

# problem constants (hardcoded per harness contract)
N, K, D = 20000, 32, 128
NCORES = 8
NS = N // NCORES  # 2500 real nodes per core
CHUNK = 128
NCH = (NS + CHUNK - 1) // CHUNK  # 20 chunks
NSP = NCH * CHUNK  # 2560 padded nodes per core
NGTOT = NCORES * NSP  # 20480 nodes in the all-gathered layer-2 source
GIDX = 8192  # layer-1 gather indices per ap_gather call
GNOD = 256  # nodes per (half-)call: 8192 idx / 32 k
IDXC = GIDX // 16  # 512 idx columns in SBUF layout per call
NCALL1 = NSP // (2 * GNOD)  # 5 calls in layer 1 (512 nodes per call)
GIDX2 = 16384  # layer-2 gather indices per call (32KB Q7 scratch: probed OK)
GNOD2 = 512  # nodes per layer-2 call
IDXC2 = GIDX2 // 16
NCALL2 = NSP // GNOD2  # 5 calls in layer 2

BF16 = mybir.dt.bfloat16
NP_BF16 = mybir.dt.np(BF16)

_COMPILED = {}


def _build(repeat: int = 1):
    f32 = mybir.dt.float32
    i16 = mybir.dt.int16
    nc = bacc.Bacc(
        "TRN2",
        target_bir_lowering=False,
        debug=False,
        enable_asserts=False,
        num_devices=NCORES,
    )
    i32 = mybir.dt.int32
    hsrc0P = nc.dram_tensor("hsrc0P", [128, N], i32, kind="ExternalInput")
    idx1 = nc.dram_tensor("idx1", [128, NCALL1, IDXC], i16, kind="ExternalInput")
    idx2 = nc.dram_tensor("idx2", [128, NCALL2, IDXC2], i16, kind="ExternalInput")
    w1mat = nc.dram_tensor("w1mat", [128, 2, D], BF16, kind="ExternalInput")
    w2mat = nc.dram_tensor("w2mat", [128, D], BF16, kind="ExternalInput")
    brep = nc.dram_tensor("brep", [128, 2, GNOD2], f32, kind="ExternalInput")
    out1 = nc.dram_tensor("out1T", [D, NSP], f32, kind="ExternalOutput")
    out2 = nc.dram_tensor("out2T", [D, NSP], f32, kind="ExternalOutput")

    with tile.TileContext(nc) as tc:
        with (
            tc.tile_pool(name="const", bufs=1) as const,
            tc.tile_pool(name="src", bufs=1) as srcp,
            tc.tile_pool(name="g", bufs=1) as gp,
            tc.tile_pool(name="gs", bufs=2) as gsp,
            tc.tile_pool(name="gt", bufs=2) as gtp,
            tc.tile_pool(name="ph", bufs=2, space="PSUM") as ph,
            tc.tile_pool(name="h", bufs=1) as hp,
            tc.tile_pool(name="dram", bufs=repeat, space="DRAM") as dram,
        ):
            idx1_sb = const.tile([128, NCALL1, IDXC], i16)
            nc.sync.dma_start(idx1_sb[:], idx1[:])
            idx2_sb = const.tile([128, NCALL2, IDXC2], i16)
            nc.sync.dma_start(idx2_sb[:], idx2[:])
            w1_sb = const.tile([128, 2, D], BF16)
            nc.sync.dma_start(w1_sb[:], w1mat[:])
            w2_sb = const.tile([128, D], BF16)
            nc.sync.dma_start(w2_sb[:], w2mat[:])
            b_sb = const.tile([128, 2, GNOD2], f32)
            nc.sync.dma_start(b_sb[:], brep[:])

            src = srcp.tile([128, NGTOT], f32)
            h1T = hp.tile([128, NSP], f32)
            h2T = hp.tile([128, NSP], f32)

            def layer1():
                for m in range(NCALL1):
                    GTbig = gp.tile([128, GIDX2], f32, tag="GT")
                    GT = GTbig[:, :GIDX]
                    nc.gpsimd.ap_gather(
                        GT,
                        src[:, :N],
                        idx1_sb[:, m, :],
                        channels=128,
                        num_elems=N,
                        d=1,
                        num_idxs=GIDX,
                    )
                    # bf16 view: per idx the pair (f=0,1); sum over k
                    gsum = gsp.tile([128, GNOD, 2], f32, tag="gsum")
                    nc.vector.tensor_reduce(
                        gsum[:],
                        GT.bitcast(BF16).rearrange(
                            "p (n k f) -> p n f k", k=K, f=2
                        ),
                        axis=mybir.AxisListType.X,
                        op=mybir.AluOpType.add,
                    )
                    # deinterleave to [p, f, n] while casting to bf16
                    gt = gtp.tile([128, 2, GNOD], BF16, tag="gt")
                    nc.scalar.copy(
                        gt[:], gsum[:].rearrange("p n f -> p f n")
                    )
                    for half, po in ((0, 0), (1, 64)):
                        phh = ph.tile([128, GNOD], f32, tag=f"phh{half}")
                        for f in range(2):
                            nc.tensor.matmul(
                                phh[:],
                                lhsT=w1_sb[po : po + 64, f, :],
                                rhs=gt[po : po + 64, f, :],
                                start=(f == 0),
                                stop=(f == 1),
                            )
                        hs = h1T[
                            :, (2 * m + half) * GNOD : (2 * m + half + 1) * GNOD
                        ]
                        nc.vector.tensor_add(hs, phh[:], b_sb[:, 0, :GNOD])
                        nc.vector.tensor_scalar_max(hs, hs, 0.0)

            def layer2():
                # 16384 idxs = 512 nodes per call, all cores on the same list
                for m in range(NCALL2):
                    GT = gp.tile([128, GIDX2], f32, tag="GT")
                    nc.gpsimd.ap_gather(
                        GT[:],
                        src[:],
                        idx2_sb[:, m, :],
                        channels=128,
                        num_elems=NGTOT,
                        d=1,
                        num_idxs=GIDX2,
                    )
                    gsum = gsp.tile([128, GNOD2], f32, tag="gsum2")
                    nc.vector.tensor_reduce(
                        gsum[:],
                        GT[:].rearrange("p (n k) -> p n k", k=K),
                        axis=mybir.AxisListType.X,
                        op=mybir.AluOpType.add,
                    )
                    gt = gtp.tile([128, GNOD2], BF16, tag="gt2")
                    nc.scalar.copy(gt[:], gsum[:])
                    phh = ph.tile([128, GNOD2], f32, tag="phh2")
                    nc.tensor.matmul(
                        phh[:],
                        lhsT=w2_sb[:],
                        rhs=gt[:],
                        start=True,
                        stop=True,
                    )
                    hs = h2T[:, m * GNOD2 : (m + 1) * GNOD2]
                    nc.vector.tensor_add(hs, phh[:], b_sb[:, 1, :])
                    nc.vector.tensor_scalar_max(hs, hs, 0.0)

            for _ in range(repeat):
                nc.sync.dma_start(src[:, :N].bitcast(i32), hsrc0P[:])
                layer1()
                ag_in = dram.tile([D, NSP], f32, tag="ag_in")
                ag_out = dram.tile(
                    [NCORES * D, NSP], f32, addr_space="Shared", tag="ag_out"
                )
                nc.sync.dma_start(ag_in[:], h1T[:])
                nc.gpsimd.collective_compute(
                    "AllGather",
                    mybir.AluOpType.bypass,
                    replica_groups=[list(range(NCORES))],
                    ins=[ag_in.opt()],
                    outs=[ag_out.opt()],
                )
                for c in range(NCORES):
                    nc.sync.dma_start(
                        src[:, c * NSP : (c + 1) * NSP],
                        ag_out[c * D : (c + 1) * D, :],
                    )
                layer2()
            nc.sync.dma_start(out1[:], h1T[:])
            nc.sync.dma_start(out2[:], h2T[:])
    nc.compile()
    return nc


def _get_compiled(repeat: int = 1):
    if repeat not in _COMPILED:
        _COMPILED[repeat] = _build(repeat)
    return _COMPILED[repeat]


def _wrap16(flat: np.ndarray) -> np.ndarray:
    """[M, GIDX] int -> 16-wrapped [16, M, IDXC] (idx i at (i%16, i//16))."""
    M = flat.shape[0]
    return flat.reshape(M, IDXC, 16).transpose(2, 0, 1).astype(np.int16)


def _idx_layout1(ix: np.ndarray) -> np.ndarray:
    """[K, NSP] ids -> layer-1 packed idx layout [128, NCALL1, IDXC].

    Call m covers nodes [512m, 512m+512): cores 0-3 (partitions 0-63) get
    the first 256 nodes' 8192 indices, cores 4-7 the next 256.
    """
    L = ix.T.reshape(NCALL1, 2, GIDX)  # [m, half, i] (i = n_off*32 + k)
    a = _wrap16(L[:, 0])  # [16, m, IDXC]
    bq = _wrap16(L[:, 1])
    return np.concatenate([np.tile(a, (4, 1, 1)), np.tile(bq, (4, 1, 1))], axis=0)


def _idx_layout2(ix: np.ndarray) -> np.ndarray:
    """[K, NSP] ids -> layer-2 idx layout [128, NCALL2, IDXC2] (all cores same)."""
    L = ix.T.reshape(NCALL2, GIDX2)
    w = L.reshape(NCALL2, IDXC2, 16).transpose(2, 0, 1).astype(np.int16)
    return np.tile(w, (8, 1, 1))


def _prep_inputs(adjacency, graph, W, b):
    adj = np.asarray(adjacency).astype(np.int64)  # [K, N]
    graph = np.asarray(graph, dtype=np.float32)  # [1, N, D]
    W = np.asarray(W, dtype=np.float32)  # [3, D, D]
    b = np.asarray(b, dtype=np.float32)  # [3, D]

    # layer-1 source, packed: partition p holds bf16 pair (2*(p%64), +1)
    h0b = graph[0].astype(NP_BF16)  # [N, 128]
    pairs = np.ascontiguousarray(
        h0b.reshape(N, 64, 2).transpose(1, 0, 2)
    )  # [64, N, 2] bf16
    pairs_u32 = pairs.view(np.uint32).reshape(64, N)
    hsrc0P = np.ascontiguousarray(np.tile(pairs_u32, (2, 1))).view(np.int32)  # [128, N]

    # W1'[p, f, e] = W[1][2*(p%64)+f, e] / K ; W2'[d, e] = W[2][d, e] / K
    w1 = (W[1] / K).astype(NP_BF16)  # [128, 128]
    w1_host = np.ascontiguousarray(
        np.tile(w1.reshape(64, 2, D), (2, 1, 1))
    )  # [128, 2, 128]
    w2_host = np.ascontiguousarray((W[2] / K).astype(NP_BF16))  # [128, 128]
    b_host = np.ascontiguousarray(
        np.broadcast_to(b[1:3].T[:, :, None], (D, 2, GNOD2))
    ).astype(np.float32)  # [128(e), 2, 256]

    jj = np.minimum(np.arange(NSP), NS - 1)  # pad nodes clamp to a real node
    in_maps = []
    for c in range(NCORES):
        ga = adj[:, NS * c + jj]  # [K, NSP] global neighbor ids
        idx1 = _idx_layout1(ga)
        idx2 = _idx_layout2((ga // NS) * NSP + (ga % NS))  # AG padded layout
        in_maps.append(
            {
                "hsrc0P": hsrc0P,
                "idx1": idx1,
                "idx2": idx2,
                "w1mat": w1_host,
                "w2mat": w2_host,
                "brep": b_host,
            }
        )
    return in_maps


def kernel(adjacency, graph, W, b):
    graph = np.asarray(graph, dtype=np.float32)
    in_maps = _prep_inputs(adjacency, graph, W, b)
    nc = _get_compiled(repeat=1)
    res = run_bass_kernel_spmd(nc, in_maps, core_ids=list(range(NCORES)), trace=False)
    h1 = np.concatenate(
        [np.asarray(res.results[c]["out1T"]).T[:NS] for c in range(NCORES)], axis=0
    )
    h2 = np.concatenate(
        [np.asarray(res.results[c]["out2T"]).T[:NS] for c in range(NCORES)], axis=0
    )
    out = np.stack([graph[0], h1, h2], axis=0)[None]  # [1, 3, N, D]
    return out.astype(np.float32)
